# revision 49
# baseline (speedup 1.0000x reference)
"""Trainium2 Bass kernel for nn_Block_18528488915394 (dense transformer block).

Strategy: 8-way sequence/batch parallelism with ZERO collectives.
  - core c (0..3) handles batch 0, token chunk [512c, 512c+512); cores 4..7
    handle batch 1 likewise.
  - Each core recomputes LN1 + K/V projection for its FULL batch (the only
    cross-token dependency is causal attention), then computes Q/attention/
    proj/MLP for its own 512-token chunk. All 16 heads per core.
  - Causality is enforced with a per-core multiplicative 0/1 mask input, so
    all 8 cores run the IDENTICAL program on different input values.

Device dataflow is feature-major ("transposed activations", [feature, token])
throughout, so every matmul contraction dim lands on SBUF partitions and no
on-chip transposes are needed:
  - LN stats (mean / mean-of-squares over features = partitions) are computed
    with an all-ones [128,128] lhsT matmul, which also broadcasts the stats
    across partitions for free.
  - softmax runs on scores^T [tok_k, tok_q]: exp on ACT, 0/1 mask on DVE,
    denominator via an all-ones lhsT matmul (broadcast across partitions),
    normalization applied on the PSUM->SBUF drain of attn@V.
  - RoPE: half-rotation done with two SBUF->SBUF partition-shift DMAs; the
    rotation sign and the 1/sqrt(hs) score scale are folded into the host-
    precomputed cos/sin tables.
Matmul operands are fp16 (full PE rate, ~5e-4 rounding) with fp32 PSUM
accumulation; LN statistics and both residual adds stay fp32.
"""
import os
import sys

import numpy as np

for _p in ("/opt/trn_rl_repo", "/root/.axon_site/_ro/trn_rl_repo"):
    if os.path.isdir(_p) and _p not in sys.path:
        sys.path.insert(0, _p)
        break

from concourse import bacc, bass, mybir, tile  # noqa: E402
from concourse import bass_utils  # noqa: E402

P = 128
C = 2048
KC = 16          # C / P
T = 2048         # tokens per batch
TQ = 512         # tokens per core chunk
HS = 128
H = 16
G = 4
HID = 8192
HC = 64          # HID / P
TC4 = 4          # T / TQ
EPS = 1e-5
B = 2
NCORES = 8

F32 = mybir.dt.float32
F16 = mybir.dt.float16
AF = mybir.ActivationFunctionType
OP = mybir.AluOpType


def _emit(nc, tc, I, out_t):
    from contextlib import ExitStack

    with ExitStack() as root:
        const = root.enter_context(tc.tile_pool(name="const", bufs=1))
        ones_mean = const.tile([P, P], F16, name="ones_mean")
        nc.vector.memset(ones_mean, 1.0 / C)   # 2^-11, exact in fp16
        ones_den = const.tile([P, P], F16, name="ones_den")
        nc.vector.memset(ones_den, 1.0)
        eps_t = const.tile([P, 1], F32, name="eps")
        nc.vector.memset(eps_t, EPS)

        stat = root.enter_context(tc.tile_pool(name="stat", bufs=2))
        sqp = root.enter_context(tc.tile_pool(name="sqp", bufs=3))

        def ln_stats(ps_pool, x_tiles, tag):
            """x_tiles: KC fp16 tiles [P, ntok]; returns f32 (mean, rstd)
            [P, ntok] with values broadcast across partitions."""
            ntok = x_tiles[0].shape[-1]
            mean_ps = ps_pool.tile([P, ntok], F32, name=f"mean_ps_{tag}")
            sq_ps = ps_pool.tile([P, ntok], F32, name=f"sq_ps_{tag}")
            for kc in range(KC):
                nc.tensor.matmul(mean_ps, ones_mean, x_tiles[kc],
                                 start=kc == 0, stop=kc == KC - 1)
            for kc in range(KC):
                sq = sqp.tile([P, ntok], F16, name="sq")
                nc.vector.tensor_mul(sq, x_tiles[kc], x_tiles[kc])
                nc.tensor.matmul(sq_ps, ones_mean, sq,
                                 start=kc == 0, stop=kc == KC - 1)
            mean_sb = stat.tile([P, ntok], F32, name="mean")
            nc.scalar.copy(mean_sb, mean_ps)
            var_sb = stat.tile([P, ntok], F32, name="var")
            nc.vector.tensor_mul(var_sb, mean_sb, mean_sb)
            nc.vector.tensor_tensor(var_sb, sq_ps, var_sb, op=OP.subtract)
            rstd = stat.tile([P, ntok], F32, name="rstd")
            nc.scalar.activation(rstd, var_sb, AF.Sqrt, bias=eps_t, scale=1.0)
            nc.vector.reciprocal_approx_fast(rstd, rstd)
            # fp16 copies for the apply step: all-fp16 operands keep the DVE
            # in its fast 16-bit mode (~2x) for the 32 apply ops per site
            mean16 = stat.tile([P, ntok], F16, name="mean16")
            nc.scalar.copy(mean16, mean_ps)
            rstd16 = stat.tile([P, ntok], F16, name="rstd16")
            nc.vector.tensor_copy(rstd16, rstd)
            return mean16, rstd16

        def ln_apply(x_tiles, mean_sb, rstd):
            for kc in range(KC):
                nc.vector.tensor_tensor(x_tiles[kc], x_tiles[kc], mean_sb,
                                        op=OP.subtract)
                nc.vector.tensor_tensor(x_tiles[kc], x_tiles[kc], rstd,
                                        op=OP.mult)

        def rope(dst, src_ps, cos_ap, sin_ap, pool):
            """dst fp16 [P, n] = raw*cos + halfshift(raw)*sin_signed."""
            n = dst.shape[-1]
            raw = pool.tile([P, n], F16, name="rp_raw")
            nc.scalar.copy(raw, src_ps)
            sh = pool.tile([P, n], F16, name="rp_sh")
            nc.sync.dma_start(sh[0:64, :], raw[64:128, :])
            nc.sync.dma_start(sh[64:128, :], raw[0:64, :])
            nc.vector.tensor_tensor(dst, raw, cos_ap, op=OP.mult)
            nc.vector.tensor_tensor(sh, sh, sin_ap, op=OP.mult)
            nc.vector.tensor_tensor(dst, dst, sh, op=OP.add)

        # PE warm-up: the HAM clock gate starts at 1.2 GHz and needs ~3.4us
        # of sustained matmul activity to release to 2.4 GHz. The first real
        # matmuls wait ~10us for input DMAs anyway, so burn that window
        # warming the PE on a zero tile (result written to out_t[0], which
        # the real epilogue overwrites later).
        with tc.tile_pool(name="warm", bufs=1) as wp, \
                tc.tile_pool(name="psW", bufs=1, space="PSUM") as psW:
            wsrc = wp.tile([P, TQ], F16, name="wsrc")
            nc.vector.memset(wsrc, 0.0)
            w_ps = psW.tile([P, TQ], F32, name="w_ps")
            for i in range(16):
                nc.tensor.matmul(w_ps, ones_mean, wsrc,
                                 start=i == 0, stop=i == 15)
            wout = wp.tile([P, 1], F32, name="wout")
            nc.scalar.copy(wout, w_ps[:, 0:1])
            nc.sync.dma_start(out_t[0, :, 0:1], wout)

        # qt lives Q..B; kt/vt live A..B. Emission order: Q first (its stats
        # matmuls fill the PE while stage A's larger DMAs stream in).
        sqt_stack = ExitStack()
        sqt = sqt_stack.enter_context(tc.tile_pool(name="sqt", bufs=1))
        qt = sqt.tile([P, H, TQ], F16, name="qt")      # roped+scaled Q^T

        # Stage-A x tiles + stats PSUM open early: chunk t+1's stat matmuls
        # are emitted between chunk t's LN-apply and its projections, so the
        # PE has queued work while the apply chain runs on the DVE.
        xcp_stack = ExitStack()
        xcp = xcp_stack.enter_context(tc.tile_pool(name="xcp", bufs=36))
        psStat_stack = ExitStack()
        psStat = psStat_stack.enter_context(
            tc.tile_pool(name="psStat", bufs=1, space="PSUM"))
        a_chunks = []

        def emit_a_stats(t):
            xcs = [xcp.tile([P, TQ], F16, name="xc") for _ in range(KC)]
            for kc in range(KC):
                nc.sync.dma_start(xcs[kc], I["xt_a"][t, :, kc, :])
            m16, r16 = ln_stats(psStat, xcs, "a")
            a_chunks.append((xcs, m16, r16))

        # ============= Stage Q: LN1(chunk) + Q =============
        with ExitStack() as sQ:
            trigq = sQ.enter_context(tc.tile_pool(name="trigq", bufs=1))
            cosq = trigq.tile([P, TQ], F16, name="cosq")
            sinq = trigq.tile([P, TQ], F16, name="sinq")
            nc.sync.dma_start(cosq, I["cos_q"])
            nc.sync.dma_start(sinq, I["sin_q"])
            xqp = sQ.enter_context(tc.tile_pool(name="xqp", bufs=18))
            wqp = sQ.enter_context(tc.tile_pool(name="wqp", bufs=4))
            rpq = sQ.enter_context(tc.tile_pool(name="rpq", bufs=2))
            psQ = sQ.enter_context(
                tc.tile_pool(name="psQ", bufs=2, space="PSUM"))

            xqs = [xqp.tile([P, TQ], F16, name="xq") for _ in range(KC)]
            for kc in range(KC):
                nc.sync.dma_start(xqs[kc], I["xt_q"][:, kc, :])
            mean_sb, rstd = ln_stats(psQ, xqs, "q")
            ln_apply(xqs, mean_sb, rstd)
            emit_a_stats(0)   # PE filler while the Q apply chain runs
            emit_a_stats(1)
            # hoist chunk-0's LN apply ahead of the q-rope DVE chains so
            # stage A's first k-proj isn't DVE-starved at the Q->A boundary
            ln_apply(*a_chunks[0][0:1], a_chunks[0][1], a_chunks[0][2])
            for h in range(H):
                wq_m = wqp.tile([P, KC, P], F16, name="wq")
                nc.sync.dma_start(wq_m, I["w_q"][h])
                q_ps = psQ.tile([P, TQ], F32, name="q_ps")
                for kc in range(KC):
                    nc.tensor.matmul(q_ps, wq_m[:, kc], xqs[kc],
                                     start=kc == 0, stop=kc == KC - 1)
                rope(qt[:, h, :], q_ps, cosq, sinq, rpq)

        # Right-side prefetch stack: maskt loads during stage A; yt/xq2/wpo
        # open later (pre-B) so their DMAs run during attention. LIFO on the
        # right stack: mkp opened first -> closes last (with yt_stack).
        yt_stack = root.enter_context(ExitStack())
        mkp = yt_stack.enter_context(
            tc.tile_pool(name="mkp", bufs=1, side="right"))
        maskt = mkp.tile([P, 6, 2 * TQ], F16, name="maskt")
        nc.sync.dma_start(maskt, I["maskt"])

        # kt/vt pool (closed explicitly after stage B)
        skv_stack = ExitStack()
        skv = skv_stack.enter_context(tc.tile_pool(name="skv", bufs=1))
        kt = skv.tile([P, G, T], F16, name="kt")        # roped K^T [hs, g, tok]
        vt = skv.tile([P, KC, G * HS], F16, name="vt")  # V [tok_sub, chunk, vcol]

        # ================= Stage A: LN1(batch) + K/V =================
        with ExitStack() as sA:
            trig = sA.enter_context(tc.tile_pool(name="trigk", bufs=1))
            cosk = trig.tile([P, T], F16, name="cosk")
            sink = trig.tile([P, T], F16, name="sink")
            nc.sync.dma_start(cosk, I["cos_k"])
            nc.sync.dma_start(sink, I["sin_k"])
            # k/v weights are small in fp16: cache them once in SBUF
            wall = sA.enter_context(tc.tile_pool(name="wall", bufs=1))
            wk_all = wall.tile([P, G, KC, P], F16, name="wk_all")
            nc.sync.dma_start(wk_all, I["w_k"])
            wv_all = wall.tile([P, KC, G * HS], F16, name="wv_all")
            nc.sync.dma_start(wv_all, I["w_v"])
            rpp = sA.enter_context(tc.tile_pool(name="rpp", bufs=2))
            psK = sA.enter_context(
                tc.tile_pool(name="psK", bufs=1, space="PSUM"))
            psV = sA.enter_context(
                tc.tile_pool(name="psV", bufs=2, space="PSUM"))

            for t in range(TC4):
                xcs, mean_sb, rstd = a_chunks[t]
                if t > 0:  # chunk 0 was applied during stage Q
                    ln_apply(xcs, mean_sb, rstd)
                if t + 2 < TC4:
                    emit_a_stats(t + 2)  # PE filler while apply(t) runs
                # K projection (kc-outer so one LN-apply feeds 4 matmuls)
                k_pss = [psK.tile([P, TQ], F32, name=f"k_ps{g}")
                         for g in range(G)]
                for kc in range(KC):
                    for g in range(G):
                        nc.tensor.matmul(k_pss[g], wk_all[:, g, kc], xcs[kc],
                                         start=kc == 0, stop=kc == KC - 1)
                for g in range(G):
                    rope(kt[:, g, t * TQ:(t + 1) * TQ], k_pss[g],
                         cosk[:, t * TQ:(t + 1) * TQ],
                         sink[:, t * TQ:(t + 1) * TQ], rpp)
                # V projection
                for i in range(4):
                    v_ps = psV.tile([P, G * HS], F32, name="v_ps")
                    for kc in range(KC):
                        nc.tensor.matmul(v_ps,
                                         xcs[kc][:, i * P:(i + 1) * P],
                                         wv_all[:, kc],
                                         start=kc == 0, stop=kc == KC - 1)
                    nc.scalar.copy(vt[:, t * 4 + i, :], v_ps)
        psStat_stack.close()

        # yt lives B..P; right-side stack so its lifetime can straddle the
        # left-stack closes of skv/sqt at end of stage B. xq2/wpo prefetch
        # pools also sit on the right so their DMAs can run during attention.
        syt = yt_stack.enter_context(
            tc.tile_pool(name="syt", bufs=1, side="right"))
        yt = syt.tile([P, H, TQ], F16, name="yt")  # normalized attn out^T
        xq2p = yt_stack.enter_context(
            tc.tile_pool(name="xq2p", bufs=18, side="right"))
        wpp = yt_stack.enter_context(
            tc.tile_pool(name="wpp", bufs=3, side="right"))
        xq2s = [xq2p.tile([P, TQ], F16, name="xq2") for _ in range(KC)]
        for kc in range(KC):
            nc.sync.dma_start(xq2s[kc], I["xt_q"][:, kc, :])

        # ============= Stage B: attention =============
        with ExitStack() as sB:
            eP = sB.enter_context(tc.tile_pool(name="eP", bufs=4))
            recP = sB.enter_context(tc.tile_pool(name="recP", bufs=2))
            psB = sB.enter_context(
                tc.tile_pool(name="psB", bufs=2, space="PSUM"))
            psS = sB.enter_context(
                tc.tile_pool(name="psS", bufs=2, space="PSUM"))

            # Each core's 512 query tokens are two 256-token chunks: A at
            # global offset 256*j (j = core's chunk index, < 1024) and B at
            # 256*(7-j) (>= 1024). K-blocks kc<8 (keys < 1024) are processed
            # for both chunks (full 512-wide rhs; B's columns are always
            # causally allowed there); blocks kc>=8 only concern chunk B's
            # columns 256:512. Score blocks share two-bank [P,1024] PSUM
            # tiles (2 full blocks, or 4 B-half blocks) so one exp + one
            # mask-multiply covers each group. This balances the causal
            # triangle across cores with an identical program on every core.
            HB = KC // 2
            pending = None  # (h, y_ps, d_ps): normalization deferred one
            # head so rec/norm DVE ops queue behind the next head's masks

            def flush_norm():
                nonlocal pending
                if pending is None:
                    return
                ph, py, pd = pending
                rec = recP.tile([P, TQ], F32, name="rec")
                nc.vector.reciprocal_approx_fast(rec, pd)
                nc.vector.tensor_tensor(yt[:, ph, :], py, rec, op=OP.mult)
                pending = None

            for h in range(H):
                g = h // 4
                y_ps = psB.tile([P, TQ], F32, name="y_ps")
                d_ps = psB.tile([P, TQ], F32, name="d_ps")
                for u in range(4):          # full-block pairs (2u, 2u+1)
                    s_ps = psS.tile([P, 2 * TQ], F32, name="s_ps")
                    for v in range(2):
                        nc.tensor.matmul(
                            s_ps[:, v * TQ:(v + 1) * TQ],
                            kt[:, g, (2 * u + v) * P:(2 * u + v + 1) * P],
                            qt[:, h, :], start=True, stop=True,
                            skip_group_check=True)
                    e_sb = eP.tile([P, 2 * TQ], F16, name="e")
                    nc.scalar.activation(e_sb, s_ps, AF.Exp)
                    nc.vector.tensor_tensor(e_sb, e_sb, maskt[:, u, :],
                                            op=OP.mult)
                    for v in range(2):
                        kc = 2 * u + v
                        ev = e_sb[:, v * TQ:(v + 1) * TQ]
                        nc.tensor.matmul(y_ps,
                                         vt[:, kc, g * HS:(g + 1) * HS],
                                         ev, start=kc == 0, stop=False,
                                         skip_group_check=True)
                        nc.tensor.matmul(d_ps, ones_den, ev,
                                         start=kc == 0, stop=False,
                                         skip_group_check=True)
                    if u == 0:
                        flush_norm()  # prev head's rec/norm after this
                        # head's first masks are already queued on the DVE
                qb = qt[:, h, 256:512]
                for u in range(2):          # B-half quads (8+4u .. 11+4u)
                    s_ps = psS.tile([P, 2 * TQ], F32, name="s_ps")
                    for v in range(4):
                        kc = HB + 4 * u + v
                        nc.tensor.matmul(
                            s_ps[:, v * 256:(v + 1) * 256],
                            kt[:, g, kc * P:(kc + 1) * P],
                            qb, start=True, stop=True, skip_group_check=True)
                    e_sb = eP.tile([P, 2 * TQ], F16, name="e")
                    nc.scalar.activation(e_sb, s_ps, AF.Exp)
                    nc.vector.tensor_tensor(e_sb, e_sb, maskt[:, 4 + u, :],
                                            op=OP.mult)
                    for v in range(4):
                        kc = HB + 4 * u + v
                        last = u == 1 and v == 3
                        ev = e_sb[:, v * 256:(v + 1) * 256]
                        nc.tensor.matmul(
                            y_ps[:, 256:512],
                            vt[:, kc, g * HS:(g + 1) * HS],
                            ev, start=False, stop=last,
                            skip_group_check=True)
                        nc.tensor.matmul(d_ps[:, 256:512], ones_den, ev,
                                         start=False, stop=last,
                                         skip_group_check=True)
                pending = (h, y_ps, d_ps)
            flush_norm()
        # left stack: close skv (kt/vt), stage-A xc pool, then sqt (qt)
        skv_stack.close()
        xcp_stack.close()
        sqt_stack.close()

        # x2 lives P..M on the left stack; x2h (fp16 copy for LN2 stats)
        # is produced incrementally during stage P. The P+M PSUM pools all
        # open here so the fc/proj pools carry no boundary dependencies.
        with tc.tile_pool(name="sx2", bufs=1) as sx2, \
                tc.tile_pool(name="sx2h", bufs=1) as sx2h, \
                tc.tile_pool(name="psM2", bufs=1, space="PSUM") as psM2, \
                tc.tile_pool(name="psF", bufs=2, space="PSUM") as psF, \
                tc.tile_pool(name="psO", bufs=2, space="PSUM") as psO:
            x2 = sx2.tile([P, KC, TQ], F32, name="x2")
            x2h = sx2h.tile([P, KC, TQ], F16, name="x2h")

            # ============= Stage P: attn proj + residual =============
            # LN2 stat matmuls are interleaved into the proj loop (x2h[m] is
            # ready per-iteration) so the stats are nearly done when P ends.
            mean_ps = psM2.tile([P, TQ], F32, name="mean_ps_m")
            sq_ps = psM2.tile([P, TQ], F32, name="sq_ps_m")
            for m in range(KC):
                wpo_m = wpp.tile([P, KC, P], F16, name="wpo")
                nc.sync.dma_start(wpo_m, I["w_po"][m])
                h_ps = psO.tile([P, TQ], F32, name="h_ps")
                for kc in range(KC):
                    nc.tensor.matmul(h_ps, wpo_m[:, kc], yt[:, kc, :],
                                     start=kc == 0, stop=kc == KC - 1)
                nc.vector.tensor_tensor(x2[:, m, :], h_ps, xq2s[m],
                                        op=OP.add)
                nc.vector.tensor_copy(x2h[:, m, :], x2[:, m, :])
                nc.tensor.matmul(mean_ps, ones_mean, x2h[:, m, :],
                                 start=m == 0, stop=m == KC - 1)
                sq = sqp.tile([P, TQ], F16, name="sq")
                nc.vector.tensor_mul(sq, x2h[:, m, :], x2h[:, m, :])
                nc.tensor.matmul(sq_ps, ones_mean, sq,
                                 start=m == 0, stop=m == KC - 1)
            yt_stack.close()  # yt/xq2/wpp (right side) released after stage P

            # ============= Stage M: LN2 + MLP =============
            # wmp/outp opened before the fc sub-scope so the mlp-proj
            # weight DMAs can prefetch during the fc matmuls.
            with ExitStack() as sM:
                gp = sM.enter_context(tc.tile_pool(name="gp", bufs=1))
                gt = gp.tile([P, HC, TQ], F16, name="gt")
                wmp = sM.enter_context(tc.tile_pool(name="wmp", bufs=2))
                outp = sM.enter_context(tc.tile_pool(name="outp", bufs=3))
                with ExitStack() as sF:
                    n2p = sF.enter_context(tc.tile_pool(name="n2p", bufs=1))
                    wfp = sF.enter_context(tc.tile_pool(name="wfp", bufs=4))
                    # LN2 stats tail (mean/sq PSUMs already accumulated)
                    mean_sb = stat.tile([P, TQ], F32, name="mean")
                    nc.scalar.copy(mean_sb, mean_ps)
                    var_sb = stat.tile([P, TQ], F32, name="var")
                    nc.vector.tensor_mul(var_sb, mean_sb, mean_sb)
                    nc.vector.tensor_tensor(var_sb, sq_ps, var_sb,
                                            op=OP.subtract)
                    rstd = stat.tile([P, TQ], F32, name="rstd")
                    nc.scalar.activation(rstd, var_sb, AF.Sqrt, bias=eps_t,
                                         scale=1.0)
                    nc.vector.reciprocal_approx_fast(rstd, rstd)
                    mean16 = stat.tile([P, TQ], F16, name="mean16")
                    nc.scalar.copy(mean16, mean_ps)
                    rstd16 = stat.tile([P, TQ], F16, name="rstd16")
                    nc.vector.tensor_copy(rstd16, rstd)
                    n2 = n2p.tile([P, KC, TQ], F16, name="n2")
                    for kc in range(KC):
                        nc.vector.tensor_tensor(n2[:, kc, :], x2h[:, kc, :],
                                                mean16, op=OP.subtract)
                        nc.vector.tensor_tensor(n2[:, kc, :], n2[:, kc, :],
                                                rstd16, op=OP.mult)
                    # first fc pair runs kc-outer so its matmuls start as
                    # soon as n2[0] exists instead of after the whole apply
                    wfc_0 = wfp.tile([P, KC, P], F16, name="wfc")
                    nc.sync.dma_start(wfc_0, I["w_fc"][0])
                    wfc_1 = wfp.tile([P, KC, P], F16, name="wfc")
                    nc.sync.dma_start(wfc_1, I["w_fc"][1])
                    f_ps0 = psF.tile([P, TQ], F32, name="f_ps")
                    f_ps1 = psF.tile([P, TQ], F32, name="f_ps")
                    for kc in range(KC):
                        nc.tensor.matmul(f_ps0, wfc_0[:, kc], n2[:, kc, :],
                                         start=kc == 0, stop=kc == KC - 1)
                        nc.tensor.matmul(f_ps1, wfc_1[:, kc], n2[:, kc, :],
                                         start=kc == 0, stop=kc == KC - 1)
                    nc.scalar.activation(gt[:, 0, :], f_ps0, AF.Gelu)
                    nc.scalar.activation(gt[:, 1, :], f_ps1, AF.Gelu)
                    for m in range(2, HC):
                        wfc_m = wfp.tile([P, KC, P], F16, name="wfc")
                        nc.sync.dma_start(wfc_m, I["w_fc"][m])
                        f_ps = psF.tile([P, TQ], F32, name="f_ps")
                        for kc in range(KC):
                            nc.tensor.matmul(f_ps, wfc_m[:, kc], n2[:, kc, :],
                                             start=kc == 0, stop=kc == KC - 1)
                        nc.scalar.activation(gt[:, m, :], f_ps, AF.Gelu)

                for m in range(KC):
                    wmp_m = wmp.tile([P, HC, P], F16, name="wmp")
                    nc.sync.dma_start(wmp_m, I["w_mp"][m])
                    o_ps = psO.tile([P, TQ], F32, name="o_ps")
                    for kh in range(HC):
                        nc.tensor.matmul(o_ps, wmp_m[:, kh], gt[:, kh, :],
                                         start=kh == 0, stop=kh == HC - 1)
                    o_sb = outp.tile([P, TQ], F32, name="o_sb")
                    nc.vector.tensor_tensor(o_sb, o_ps, x2[:, m, :],
                                            op=OP.add)
                    nc.sync.dma_start(out_t[m], o_sb)


_INPUT_SPECS = [
    ("xt_a", [TC4, P, KC, TQ], F16),
    ("xt_q", [P, KC, TQ], F16),
    ("maskt", [P, 6, 2 * TQ], F16),
    ("cos_k", [P, T], F16),
    ("sin_k", [P, T], F16),
    ("cos_q", [P, TQ], F16),
    ("sin_q", [P, TQ], F16),
    ("w_q", [H, P, KC, P], F16),
    ("w_k", [P, G, KC, P], F16),
    ("w_v", [P, KC, G * HS], F16),
    ("w_po", [KC, P, KC, P], F16),
    ("w_fc", [HC, P, KC, P], F16),
    ("w_mp", [KC, P, HC, P], F16),
]

_NC_CACHE = {}


def build_nc():
    if "nc" in _NC_CACHE:
        return _NC_CACHE["nc"]
    nc = bacc.Bacc("TRN2", target_bir_lowering=False, debug=False,
                   enable_asserts=False, num_devices=NCORES)
    ins = {}
    for name, shape, dt in _INPUT_SPECS:
        ins[name] = nc.dram_tensor(name, shape, dt, kind="ExternalInput").ap()
    out_t = nc.dram_tensor("outt", [KC, P, TQ], F32, kind="ExternalOutput").ap()
    with tile.TileContext(nc) as tc:
        _emit(nc, tc, ins, out_t)
    nc.compile()
    _NC_CACHE["nc"] = nc
    return nc


def _tile_lhsT(w, mc, kc=KC):
    """W [K, M] -> [MC, P, KC, P] lhsT tiles: [m, p, k, j] = W[k*P+p, m*P+j]."""
    K, M = w.shape
    assert K == kc * P and M == mc * P
    return np.ascontiguousarray(
        w.reshape(kc, P, mc, P).transpose(2, 1, 0, 3))


def prepare_in_maps(inputs):
    x = np.asarray(inputs["x"], np.float32)
    cos = np.asarray(inputs["cos"], np.float32)
    sin = np.asarray(inputs["sin"], np.float32)
    mask = np.asarray(inputs["mask"])
    w_attn = np.asarray(inputs["w_attn"], np.float32)
    # Fold LN elementwise weights into the consuming projections (these are
    # ones/zeros per the problem spec; folding the scale keeps generality).
    ln1_w = np.asarray(inputs["ln1_w"], np.float32)
    ln2_w = np.asarray(inputs["ln2_w"], np.float32)

    wa = w_attn.reshape(C, G, 6, HS)
    wq = wa[:, :, :4].reshape(C, H * HS) * ln1_w[:, None]
    wk = wa[:, :, 4].reshape(C, G * HS) * ln1_w[:, None]
    wv = wa[:, :, 5].reshape(C, G * HS) * ln1_w[:, None]
    wpo = np.asarray(inputs["w_proj"], np.float32)
    wfc = np.asarray(inputs["w_fc"], np.float32) * ln2_w[:, None]
    wmp = np.asarray(inputs["w_mlp_proj"], np.float32)

    f16 = np.float16
    sgn = np.concatenate([-np.ones(64, np.float32), np.ones(64, np.float32)])
    scale = np.float32(1.0 / np.sqrt(HS))
    shared = {
        "cos_k": np.ascontiguousarray(cos.T).astype(f16),
        "sin_k": np.ascontiguousarray(sin.T * sgn[:, None]).astype(f16),
        "w_q": _tile_lhsT(wq, H).astype(f16),
        "w_k": np.ascontiguousarray(
            _tile_lhsT(wk, G).transpose(1, 0, 2, 3)).astype(f16),
        "w_v": np.ascontiguousarray(
            wv.reshape(KC, P, G * HS).transpose(1, 0, 2)).astype(f16),
        "w_po": _tile_lhsT(wpo, KC).astype(f16),
        "w_fc": _tile_lhsT(wfc, HC).astype(f16),
        "w_mp": _tile_lhsT(wmp, KC, kc=HC).astype(f16),
    }
    xt_a = {}
    for b in range(B):
        xt_a[b] = np.ascontiguousarray(
            x[b].T.reshape(KC, P, TC4, TQ).transpose(2, 1, 0, 3)).astype(f16)
    in_maps = []
    for core in range(NCORES):
        b, toks = _core_tokens(core)
        xq = np.ascontiguousarray(
            x[b, toks].T.reshape(KC, P, TQ).transpose(1, 0, 2))
        base = mask[0, 0, toks, :].T.reshape(KC, P, TQ) \
            .transpose(1, 0, 2).astype(f16)       # [P, KC, TQ]
        mt = np.empty((P, 6, 2 * TQ), f16)
        for u in range(4):  # full-block pairs (2u, 2u+1)
            mt[:, u, :TQ] = base[:, 2 * u, :]
            mt[:, u, TQ:] = base[:, 2 * u + 1, :]
        for u in range(2):  # B-half quads (8+4u .. 11+4u)
            for v in range(4):
                mt[:, 4 + u, v * 256:(v + 1) * 256] = \
                    base[:, 8 + 4 * u + v, 256:512]
        mt = np.ascontiguousarray(mt)
        m = dict(shared)
        m["xt_a"] = xt_a[b]
        m["xt_q"] = xq.astype(f16)
        m["maskt"] = mt
        m["cos_q"] = np.ascontiguousarray(cos[toks].T * scale).astype(f16)
        m["sin_q"] = np.ascontiguousarray(
            sin[toks].T * sgn[:, None] * scale).astype(f16)
        in_maps.append(m)
    return in_maps


def _core_tokens(core):
    """Core -> (batch, 512 global token positions): chunks 256*j and
    256*(7-j) — balances the causal-attention triangle across cores."""
    b, j = core // 4, core % 4
    offa, offb = 256 * j, 256 * (7 - j)
    toks = np.concatenate([np.arange(offa, offa + 256),
                           np.arange(offb, offb + 256)])
    return b, toks


def run(inputs, trace=False, **kwargs):
    nc = build_nc()
    in_maps = prepare_in_maps(inputs)
    res = bass_utils.run_bass_kernel_spmd(
        nc, in_maps, core_ids=list(range(NCORES)), trace=trace, **kwargs)
    out = np.empty((B, T, C), dtype=np.float32)
    for core in range(NCORES):
        b, toks = _core_tokens(core)
        ot = res.results[core]["outt"]  # [KC, P, TQ]
        out[b, toks] = ot.transpose(2, 0, 1).reshape(TQ, C)
    return out, res


def kernel(**inputs):
    out, _ = run(inputs, trace=False)
    return out
